# revision 2
# baseline (speedup 1.0000x reference)
"""Bass/Tile TRN2 kernel for nn_DecomposedRotateNet (dense_mlp).

Contract: kernel(**inputs) takes FULL unsharded numpy inputs (as produced by
setup_inputs()) and returns the FULL [4096, 64] float32 output.

Strategy: pure data parallel over 8 NeuronCores — batch 4096 -> 512 rows/core,
small MLP weights replicated. All layout prep (transposes, bias folding,
broadcast of LN affine rows) happens on host; the device kernel is pure
compute. Activations for the dominant index-net phase are kept in
[feature, row] layout so every layer is a plain PE matmul, and the
positional one-hot term is applied as a per-partition ACT bias broadcast.
"""

import os
import sys
import functools

import numpy as np

for _p in ("/opt/trn_rl_repo",):
    if _p not in sys.path and os.path.isdir(_p):
        sys.path.insert(0, _p)

import concourse.bacc as bacc
import concourse.bass as bass
import concourse.mybir as mybir
import concourse.tile as tile
from concourse import bass_utils
from concourse._compat import with_exitstack
from contextlib import ExitStack

B, BITS, HID = 4096, 64, 512
NCORES = 8
BC = B // NCORES          # 512 batch rows per core
NBT = BC // 128           # 4 batch tiles of 128 (phase 1)
NKC = HID // 128          # 4 chunks of the hidden dim
EPS = 1e-5

F32 = mybir.dt.float32
F32R = mybir.dt.float32r

# Matmul operand dtype for the heavy phase-2 matmuls and phase-1 matmuls.
# float32r/bf16 stream at 1 cycle/row (vs 4 for fp32) on TRN2's PE.
_MM_DT_NAME = os.environ.get("KERNEL_MM_DT", "bf16")
MM_DT = {
    "f32r": F32R,
    "f32": F32,
    "bf16": mybir.dt.bfloat16,
}[_MM_DT_NAME]


def _mm(nc, out, lhsT, rhs, start, stop, dt=None):
    if dt is not None:
        lhsT = lhsT.bitcast(dt)
        rhs = rhs.bitcast(dt)
    nc.tensor.matmul(out, lhsT, rhs, start=start, stop=stop)


@with_exitstack
def _build_kernel(ctx: ExitStack, tc: "tile.TileContext", io: dict):
    nc = tc.nc
    AF = mybir.ActivationFunctionType
    ALU = mybir.AluOpType

    persist = ctx.enter_context(tc.tile_pool(name="persist", bufs=1))

    def load(name, shape, dt=F32):
        t = persist.tile(shape, dt, name=f"sb_{name}", tag=f"sb_{name}")
        nc.sync.dma_start(t[:], io[name][:])
        return t

    # ---- persistent SBUF tensors --------------------------------------
    aT = load("aT", [BITS, BC], MM_DT)            # a_bits shard, transposed
    w1t = load("w1t", [BITS + 1, HID], MM_DT)     # [W1.T ; b1]
    w2t = [load(f"w2t{i}", [128, HID], MM_DT) for i in range(NKC)]
    w2b = load("w2b", [1, HID], MM_DT)
    w3t = [load(f"w3t{i}", [128, BITS], MM_DT) for i in range(NKC)]
    w3b = load("w3b", [1, BITS], MM_DT)
    g1bc = load("g1bc", [128, HID])
    be1bc = load("be1bc", [128, HID])
    g2bc = load("g2bc", [128, HID])
    be2bc = load("be2bc", [128, HID])
    wi1bt = load("wi1bt", [BITS, HID], MM_DT)     # Wi1[:, 64:].T
    posb = [load(f"posb{i}", [128, BITS]) for i in range(NKC)]  # Wi1[:,:64]+bi1
    wi2t = [load(f"wi2t{i}", [128, HID], MM_DT) for i in range(NKC)]   # Wi2.T chunks
    bi2c = load("bi2c", [128, NKC])        # bi2 as 4 columns of 128
    wi3t = [load(f"wi3t{i}", [128, BITS], MM_DT) for i in range(NKC)]  # Wi3.T chunks
    bi3c = load("bi3c", [BITS, 1])
    ident = load("ident", [128, 128])
    ones64 = load("ones64", [BITS, 1], MM_DT)

    # x0a = [shift_bits_T shard ; ones row] for the z1 matmul (bias fold)
    x0a = persist.tile([BITS + 1, BC], MM_DT, name="x0a", tag="x0a")
    nc.sync.dma_start(x0a[0:BITS, :], io["sbT"][:])
    nc.sync.dma_start(x0a[BITS : BITS + 1, :], io["onesr"][:])

    epsc = persist.tile([128, 1], F32, name="epsc", tag="epsc")
    nc.vector.memset(epsc[:], EPS)
    ones1r = persist.tile([1, BC], MM_DT, name="ones1r", tag="ones1r")
    nc.sync.dma_start(ones1r[:], io["onesr"][:])

    ssT = persist.tile([BITS, BC], MM_DT, name="ssT", tag="ssT")      # shift_soft.T
    shiftT = [
        persist.tile([128, BC], MM_DT, name=f"shiftT{i}", tag=f"shiftT{i}")
        for i in range(NKC)
    ]

    # =================== phase 1: shift decoder =======================
    with (
        tc.tile_pool(name="p1s", bufs=3) as p1s,
        tc.tile_pool(name="p1p", bufs=2, space="PSUM") as p1p,
    ):
        def layernorm_relu(z_psum, gbc, bebc, out_tag):
            """z [128, 512] PSUM -> relu(LN(z)*g+be) [128, 512] SBUF."""
            stats = p1s.tile([128, 6], F32, tag="stats")
            nc.vector.bn_stats(stats[:], z_psum[:])
            mv = p1s.tile([128, 2], F32, tag="mv")
            nc.vector.bn_aggr(mv[:], stats[:])
            std = p1s.tile([128, 1], F32, tag="std")
            nc.scalar.activation(std[:], mv[:, 1:2], AF.Sqrt, bias=epsc[:])
            rinv = p1s.tile([128, 1], F32, tag="rinv")
            nc.vector.reciprocal(rinv[:], std[:])
            nmr = p1s.tile([128, 1], F32, tag="nmr")
            # nmr = (mean * -1) * rinv
            nc.vector.scalar_tensor_tensor(
                nmr[:], mv[:, 0:1], -1.0, rinv[:], op0=ALU.mult, op1=ALU.mult
            )
            xn = p1s.tile([128, HID], F32, tag="xn")
            nc.scalar.activation(xn[:], z_psum[:], AF.Identity, bias=nmr[:], scale=rinv[:])
            t1 = p1s.tile([128, HID], F32, tag="t1")
            nc.vector.tensor_tensor(t1[:], xn[:], gbc[:], op=ALU.mult)
            t2 = p1s.tile([128, HID], F32, tag="t2")
            nc.vector.tensor_tensor(t2[:], t1[:], bebc[:], op=ALU.add)
            a = p1s.tile([128, HID], F32, tag=out_tag)
            nc.vector.tensor_scalar_max(a[:], t2[:], 0.0)
            return a

        def transpose128(src, cols, out_tag):
            """src [128, cols*128] SBUF -> list of [128,128] SBUF transposed chunks."""
            outs = []
            for h in range(cols):
                tp = p1p.tile([128, 128], F32, tag="tp")
                nc.tensor.transpose(tp[:], src[:, h * 128 : (h + 1) * 128], ident[:])
                sb = p1s.tile([128, 128], MM_DT, tag=f"{out_tag}{h}")
                nc.scalar.copy(sb[:], tp[:])
                outs.append(sb)
            return outs

        for bt in range(NBT):
            bs = slice(bt * 128, (bt + 1) * 128)
            ones1 = ones1r[:, bs]

            z1 = p1p.tile([128, HID], F32, tag="zz", bufs=3)
            _mm(nc, z1[:], x0a[:, bs], w1t[:], True, True)
            a1 = layernorm_relu(z1, g1bc, be1bc, "a1")
            a1T = transpose128(a1, NKC, "a1T")

            z2 = p1p.tile([128, HID], F32, tag="zz", bufs=3)
            for h in range(NKC):
                _mm(nc, z2[:], a1T[h][:], w2t[h][:], h == 0, False)
            _mm(nc, z2[:], ones1, w2b[:], False, True)
            a2 = layernorm_relu(z2, g2bc, be2bc, "a2")
            a2T = transpose128(a2, NKC, "a2T")

            z3 = p1p.tile([128, BITS], F32, tag="z3")
            for h in range(NKC):
                _mm(nc, z3[:], a2T[h][:], w3t[h][:], h == 0, False)
            _mm(nc, z3[:], ones1, w3b[:], False, True)

            # softmax over free dim (64)
            mx = p1s.tile([128, 1], F32, tag="mx")
            nc.vector.reduce_max(mx[:], z3[:], axis=mybir.AxisListType.X)
            nmx = p1s.tile([128, 1], F32, tag="nmx")
            nc.vector.tensor_scalar_mul(nmx[:], mx[:], -1.0)
            es = p1s.tile([128, BITS], F32, tag="es")
            ssum = p1s.tile([128, 1], F32, tag="ssum")
            nc.scalar.activation(
                es[:], z3[:], AF.Exp, bias=nmx[:], accum_out=ssum[:]
            )
            rs = p1s.tile([128, 1], F32, tag="rs")
            nc.vector.reciprocal(rs[:], ssum[:])
            ss = p1s.tile([128, BITS], F32, tag="ss")
            nc.vector.tensor_scalar_mul(ss[:], es[:], rs[:])

            # transpose shift_soft tile -> ssT[:, bt]
            tps = p1p.tile([BITS, 128], F32, tag="tp")
            nc.tensor.transpose(tps[:], ss[:], ident[:])
            nc.scalar.copy(ssT[:, bs], tps[:])

        # ---- phase 1.5: shiftT = Wi1[:,64:] @ shift_soft.T ----------
        for fc in range(NKC):
            sp = p1p.tile([128, BC], F32, tag="zz", bufs=3)
            _mm(nc, sp[:], wi1bt[:, fc * 128 : (fc + 1) * 128], ssT[:], True, True)
            nc.scalar.copy(shiftT[fc][:], sp[:])

    # =================== phase 2: index net ===========================
    # row-tile t = output position p=t over all 512 batch cols.
    # 2-deep software pipeline so PE never waits on ACT/DVE epilogues.
    with (
        tc.tile_pool(name="p2s", bufs=2) as p2s,
        tc.tile_pool(name="p2e", bufs=3) as p2e,
        tc.tile_pool(name="p2p", bufs=1, space="PSUM") as p2p,
        tc.tile_pool(name="p2pl", bufs=2, space="PSUM") as p2pl,
    ):
        NP = BITS  # 64 row tiles

        st = {}  # per-stage state

        def stage_h1(t):
            """h1 build (ACT broadcast of position column)."""
            h1 = [
                p2s.tile([128, BC], MM_DT, tag=f"h1_{fc}", name=f"h1_{fc}")
                for fc in range(NKC)
            ]
            for fc in range(NKC):
                nc.scalar.activation(
                    h1[fc][:],
                    shiftT[fc][:],
                    mybir.ActivationFunctionType.Relu,
                    bias=posb[fc][:, t : t + 1],
                )
            st[t] = {"h1": h1}

        def stage_a(t):
            """MM1 + relu -> h2."""
            h1 = st[t]["h1"]
            h2 = [
                p2s.tile([128, BC], MM_DT, tag=f"h2_{kc}", name=f"h2_{kc}")
                for kc in range(NKC)
            ]
            for kc in range(NKC):
                z = p2p.tile([128, BC], F32, tag=f"h2z{kc}")
                for fc in range(NKC):
                    _mm(
                        nc, z[:],
                        wi2t[fc][:, kc * 128 : (kc + 1) * 128],
                        h1[fc][:],
                        fc == 0, fc == NKC - 1,
                    )
                nc.vector.tensor_scalar(
                    h2[kc][:],
                    z[:],
                    bi2c[:, kc : kc + 1],
                    0.0,
                    op0=mybir.AluOpType.add,
                    op1=mybir.AluOpType.max,
                )
            st[t]["h2"] = h2

        def stage_b(t):
            """MM2 + exp + a-weighting."""
            h2 = st[t]["h2"]
            lg = p2pl.tile([BITS, BC], F32, tag="lg")
            for kc in range(NKC):
                _mm(
                    nc, lg[:],
                    wi3t[kc][:],
                    h2[kc][:],
                    kc == 0, kc == NKC - 1, MM_DT,
                )
            e = p2e.tile([BITS, BC], MM_DT, tag="e")
            nc.scalar.activation(
                e[:], lg[:], mybir.ActivationFunctionType.Exp, bias=bi3c[:]
            )
            tmp = p2e.tile([BITS, BC], MM_DT, tag="tmp")
            nc.vector.tensor_tensor(tmp[:], e[:], aT[:], op=mybir.AluOpType.mult)
            st[t]["e"], st[t]["tmp"] = e, tmp

        def stage_c(t):
            """colsums + normalize -> outT row t."""
            e, tmp = st[t]["e"], st[t]["tmp"]
            s = p2pl.tile([1, BC], F32, tag="s", bufs=1)
            _mm(nc, s[:], ones64[:], e[:], True, True)
            d = p2pl.tile([1, BC], F32, tag="d", bufs=1)
            _mm(nc, d[:], ones64[:], tmp[:], True, True)
            r = p2e.tile([1, BC], F32, tag="r")
            nc.vector.reciprocal(r[:], s[:])
            orow = p2e.tile([1, BC], F32, tag="orow")
            nc.vector.tensor_tensor(orow[:], d[:], r[:], op=mybir.AluOpType.mult)
            nc.sync.dma_start(io["out_t"][t : t + 1, :], orow[:])
            del st[t]

        for t in range(NP):
            stage_h1(t)
            if t >= 1:
                stage_b(t - 1)
            stage_a(t)
            if t >= 2:
                stage_c(t - 2)
        stage_b(NP - 1)
        stage_c(NP - 2)
        stage_c(NP - 1)


_MM_INPUTS = frozenset(
    ["sbT", "aT", "w1t", "w2b", "w3b", "wi1bt", "ones64", "onesr"]
    + [f"w2t{i}" for i in range(NKC)]
    + [f"w3t{i}" for i in range(NKC)]
    + [f"wi2t{i}" for i in range(NKC)]
    + [f"wi3t{i}" for i in range(NKC)]
)

_INPUT_SPECS = [
    ("sbT", [BITS, BC]),
    ("aT", [BITS, BC]),
    ("w1t", [BITS + 1, HID]),
    *[(f"w2t{i}", [128, HID]) for i in range(NKC)],
    ("w2b", [1, HID]),
    *[(f"w3t{i}", [128, BITS]) for i in range(NKC)],
    ("w3b", [1, BITS]),
    ("g1bc", [128, HID]),
    ("be1bc", [128, HID]),
    ("g2bc", [128, HID]),
    ("be2bc", [128, HID]),
    ("wi1bt", [BITS, HID]),
    *[(f"posb{i}", [128, BITS]) for i in range(NKC)],
    *[(f"wi2t{i}", [128, HID]) for i in range(NKC)],
    ("bi2c", [128, NKC]),
    *[(f"wi3t{i}", [128, BITS]) for i in range(NKC)],
    ("bi3c", [BITS, 1]),
    ("ident", [128, 128]),
    ("ones64", [BITS, 1]),
    ("onesr", [1, BC]),
]


@functools.lru_cache(maxsize=1)
def _get_nc():
    nc = bacc.Bacc("TRN2", target_bir_lowering=False, debug=False, num_devices=NCORES)
    io = {}
    for name, shape in _INPUT_SPECS:
        dt = MM_DT if name in _MM_INPUTS else F32
        io[name] = nc.dram_tensor(name, shape, dt, kind="ExternalInput").ap()
    io["out_t"] = nc.dram_tensor("out_t", [BITS, BC], F32, kind="ExternalOutput").ap()
    with tile.TileContext(nc) as tc:
        _build_kernel(tc, io)
    nc.compile()
    return nc


def _host_prep(inputs):
    """Shared (replicated) weight-derived tensors, all float32 numpy."""
    f = lambda x: np.ascontiguousarray(np.asarray(x, dtype=np.float32))
    W1, b1 = f(inputs["W1"]), f(inputs["b1"])
    W2, b2 = f(inputs["W2"]), f(inputs["b2"])
    W3, b3 = f(inputs["W3"]), f(inputs["b3"])
    Wi1, bi1 = f(inputs["Wi1"]), f(inputs["bi1"])
    Wi2, bi2 = f(inputs["Wi2"]), f(inputs["bi2"])
    Wi3, bi3 = f(inputs["Wi3"]), f(inputs["bi3"])
    g1, be1 = f(inputs["g1"]), f(inputs["be1"])
    g2, be2 = f(inputs["g2"]), f(inputs["be2"])

    shared = {}
    shared["w1t"] = np.vstack([W1.T, b1[None, :]])
    w2t_full = W2.T
    for i in range(NKC):
        shared[f"w2t{i}"] = np.ascontiguousarray(w2t_full[i * 128 : (i + 1) * 128])
    shared["w2b"] = b2[None, :]
    w3t_full = W3.T
    for i in range(NKC):
        shared[f"w3t{i}"] = np.ascontiguousarray(w3t_full[i * 128 : (i + 1) * 128])
    shared["w3b"] = b3[None, :]
    shared["g1bc"] = np.broadcast_to(g1[None, :], (128, HID)).copy()
    shared["be1bc"] = np.broadcast_to(be1[None, :], (128, HID)).copy()
    shared["g2bc"] = np.broadcast_to(g2[None, :], (128, HID)).copy()
    shared["be2bc"] = np.broadcast_to(be2[None, :], (128, HID)).copy()
    shared["wi1bt"] = np.ascontiguousarray(Wi1[:, BITS:].T)
    posb_full = Wi1[:, :BITS] + bi1[:, None]
    for i in range(NKC):
        shared[f"posb{i}"] = np.ascontiguousarray(posb_full[i * 128 : (i + 1) * 128])
    wi2t_full = Wi2.T
    for i in range(NKC):
        shared[f"wi2t{i}"] = np.ascontiguousarray(wi2t_full[i * 128 : (i + 1) * 128])
    shared["bi2c"] = np.ascontiguousarray(bi2.reshape(NKC, 128).T)
    wi3t_full = Wi3.T
    for i in range(NKC):
        shared[f"wi3t{i}"] = np.ascontiguousarray(wi3t_full[i * 128 : (i + 1) * 128])
    shared["bi3c"] = bi3[:, None]
    shared["ident"] = np.eye(128, dtype=np.float32)
    shared["ones64"] = np.ones((BITS, 1), dtype=np.float32)
    shared["onesr"] = np.ones((1, BC), dtype=np.float32)
    return shared


def _mm_np_dtype():
    import concourse.mybir as _mybir

    return _mybir.dt.np(MM_DT)


def _make_in_maps(inputs):
    shared = _host_prep(inputs)
    mmdt = _mm_np_dtype()
    shared = {
        k: (v.astype(mmdt) if k in _MM_INPUTS else v) for k, v in shared.items()
    }
    a_bits = np.asarray(inputs["a_bits"], dtype=np.float32)
    shift_bits = np.asarray(inputs["shift_bits"], dtype=np.float32)
    in_maps = []
    for c in range(NCORES):
        rows = slice(c * BC, (c + 1) * BC)
        m = dict(shared)
        m["sbT"] = np.ascontiguousarray(shift_bits[rows].T).astype(mmdt)
        m["aT"] = np.ascontiguousarray(a_bits[rows].T).astype(mmdt)
        in_maps.append(m)
    return in_maps


def assemble_output(results):
    out = np.empty((B, BITS), dtype=np.float32)
    for c in range(NCORES):
        out[c * BC : (c + 1) * BC] = results[c]["out_t"].T
    return out


def run_on_cores(inputs, trace=False):
    """Returns (full_output [4096, 64] f32, BassKernelResults)."""
    nc = _get_nc()
    in_maps = _make_in_maps(inputs)
    res = bass_utils.run_bass_kernel_spmd(
        nc, in_maps, list(range(NCORES)), trace=trace
    )
    out = np.empty((B, BITS), dtype=np.float32)
    for c in range(NCORES):
        out[c * BC : (c + 1) * BC] = res.results[c]["out_t"].T
    return out, res


def kernel(**inputs) -> np.ndarray:
    out, _ = run_on_cores(inputs, trace=False)
    return out

